# revision 18
# baseline (speedup 1.0000x reference)
"""LocalPoolPointnet on 8 Trainium2 cores (v5: segmented scans, no one-hot).

The data has ~124 occupied bins per core with 30-800 points each (points
sorted by bin).  scatter_mean + gather become three DVE passes per tile:

  fs  = tensor_tensor_scan(m, net)        # segmented running sum (m: 0 at
                                          # segment starts, else 1)
  fs *= em_rp                             # keep mean (= sum * 1/cnt) at each
                                          # segment END, zero elsewhere
  tot = reverse-scan(m>>1, fs)            # broadcast each segment's mean
                                          # back over the segment (hold scan)

All matmuls are plain fp32 resblock GEMMs; there is no bin table, no one-hot,
no transposes, no dynamic addressing on the device.  Points are packed into
static REGIONS (whole bins per region, dummy-padded); scans chain inside a
region via static initial wiring and the masks kill any cross-region carry.
The head emits masked means in point space; the host gathers segment ends.
"""

import sys
import numpy as np

# ---------------------------------------------------------------- constants
B = 2
NP_ = 100_000
HID = 128
D2 = 256
NBLK = 5
RES = 64
R = 20_005          # max_coord_num in the reference
BIG = RES ** 3 + 1
NCORES = 8
CORES_PER_BATCH = NCORES // B

NCHUNK = 52                     # 512-pt chunks per core
NPTS = NCHUNK * 512             # 26624 padded points per core
REGIONS = [6, 6, 6, 6, 6, 6, 6, 6, 4]   # chunks per region (sum = 52)
assert sum(REGIONS) == NCHUNK
NBIAS = 13                      # b_pos(2) b0(5) b1(5) b_c(1)
FW = NPTS * 4 // 128            # 832

F32 = np.float32
F16 = np.float16


# ================================================================ host prep
def point_meta(p, sparse_coords, res):
    """Integer routing metadata, bit-identical to the reference's indexing."""
    p = np.asarray(p, F32)
    sc = np.asarray(sparse_coords)
    coord = np.clip(p + F32(0.5), F32(1e-6), F32(1.0 - 1e-6)) * F32(res)
    cl = coord.astype(np.int32)
    lin = (cl[..., 0] * res + cl[..., 1]) * res + cl[..., 2]      # [B, NP]

    slin = (sc[:, 1] * res + sc[:, 2]) * res + sc[:, 3]
    index = np.empty((B, NP_), np.int64)
    for b in range(B):
        coords_b = np.sort(np.where(sc[:, 0] == b, slin, BIG))
        index[b] = np.searchsorted(coords_b, lin[b], side="left")
    counts = np.bincount(sc[:, 0], minlength=B)
    return index, counts


def shard(p, index):
    """Split each batch's points into CORES_PER_BATCH contiguous-bin shards."""
    shards = []
    for b in range(B):
        idx = index[b]
        order = np.argsort(idx, kind="stable")
        sidx = idx[order]
        binc = np.bincount(idx, minlength=R)
        csum = np.cumsum(binc)
        prev_hi = 0
        for c in range(CORES_PER_BATCH):
            if c < CORES_PER_BATCH - 1:
                target = (c + 1) * NP_ // CORES_PER_BATCH
                hi = int(np.searchsorted(csum, target))
                if hi > 0 and target - csum[hi - 1] < csum[hi] - target:
                    hi -= 1
                hi += 1          # shard owns bins [lo, hi)
            else:
                hi = R
            lo = prev_hi
            prev_hi = hi
            sel = slice(int(np.searchsorted(sidx, lo)), int(np.searchsorted(sidx, hi)))
            pts = p[b][order[sel]]                     # [n, 3] sorted by bin
            rel = (sidx[sel] - lo).astype(np.int64)    # sorted rel bins
            shards.append(dict(batch=b, lo=lo, hi=hi, pts=pts, rel=rel,
                               nb=hi - lo))
    return shards


def core_inputs(sh):
    """Whole-bin region packing + scan masks for one core."""
    n = sh["pts"].shape[0]
    rel = sh["rel"]

    pts = np.full((NPTS, 3), 0.25, F32)
    m = np.ones(NPTS + 16, F32)          # scan carry mask (0 = segment start)
    emrp = np.zeros(NPTS, F32)           # 1/cnt at segment ends, else 0
    end_pos = {}                         # rel bin -> padded end position

    # bin run boundaries
    starts = np.flatnonzero(np.r_[True, rel[1:] != rel[:-1]])
    ends = np.r_[starts[1:], n]
    nbin = len(starts)

    region_caps = [r * 512 for r in REGIONS]
    # best-fit-decreasing bin packing into regions (bin order is free)
    order = sorted(range(nbin), key=lambda k: -(ends[k] - starts[k]))
    left = list(region_caps)
    fill = [0] * len(region_caps)
    rbase = np.cumsum([0] + region_caps[:-1])
    assign = {}
    for k in order:
        ln = int(ends[k] - starts[k])
        cands = [r for r in range(len(left)) if left[r] >= ln]
        assert cands, f"bin of {ln} pts does not fit any region"
        r = min(cands, key=lambda r_: left[r_])
        assign[k] = r
        left[r] -= ln
    for k in range(nbin):
        r = assign[k]
        s, e = starts[k], ends[k]
        ln = e - s
        pos = int(rbase[r]) + fill[r]
        pts[pos:pos + ln] = sh["pts"][s:e]
        m[pos] = 0.0
        emrp[pos + ln - 1] = F32(1.0) / F32(ln)
        end_pos[int(rel[s])] = pos + ln - 1
        fill[r] += ln
    m[0] = 0.0
    # region starts always begin a new segment (kills static scan carry)
    off = 0
    for cap in region_caps:
        m[off] = 0.0
        off += cap

    cnt = np.bincount(rel, minlength=max(sh["nb"], 1)).astype(F32)

    pts4 = np.zeros((4, NPTS), F32)
    pts4[:3] = pts.T
    pts_flat = np.ascontiguousarray(pts4).reshape(128, FW)
    return dict(pts_flat=pts_flat, m_row=m[None, :].astype(F16),
                emrp=emrp.reshape(NCHUNK, 512), cnt=cnt, end_pos=end_pos)


def weight_inputs(W_pos, b_pos, W0, b0, W1, b1, Ws, Wc, b_c):
    W_pos, W0, W1, Ws, Wc = [np.ascontiguousarray(x, F32)
                             for x in (W_pos, W0, W1, Ws, Wc)]
    wpos4 = np.zeros((4, D2), F32)
    wpos4[:3] = W_pos
    bias = np.zeros((128, NBIAS), F32)
    bias[:, 0] = np.asarray(b_pos, F32)[:128]
    bias[:, 1] = np.asarray(b_pos, F32)[128:]
    bias[:, 2:7] = np.asarray(b0, F32).T
    bias[:, 7:12] = np.asarray(b1, F32).T
    bias[:, 12] = np.asarray(b_c, F32)
    return dict(wpos4=wpos4, w0=W0, w1=W1, ws=Ws, wc=Wc, bias=bias)


# ================================================================ bass build
def build_bass():
    if "/opt/trn_rl_repo" not in sys.path:
        sys.path.insert(0, "/opt/trn_rl_repo")
    import concourse.mybir as mybir
    from concourse import bacc, tile
    from contextlib import ExitStack

    dt = mybir.dt.float32
    dh = mybir.dt.float16
    AF = mybir.ActivationFunctionType
    OP = mybir.AluOpType
    GELU = AF.Gelu_apprx_tanh

    # region layout in chunks
    rbounds = []
    off = 0
    for r in REGIONS:
        rbounds.append((off, off + r))
        off += r

    nc = bacc.Bacc("TRN2")
    # -------- dram io
    d_pts = nc.dram_tensor("pts_flat", [128, FW], dt, kind="ExternalInput")
    d_m = nc.dram_tensor("m_row", [1, NPTS + 16], dh, kind="ExternalInput")
    d_emr = nc.dram_tensor("emrp", [NCHUNK, 512], dt, kind="ExternalInput")
    d_wpos4 = nc.dram_tensor("wpos4", [4, D2], dt, kind="ExternalInput")
    d_w0 = nc.dram_tensor("w0", [NBLK, D2, HID], dt, kind="ExternalInput")
    d_w1 = nc.dram_tensor("w1", [NBLK, HID, HID], dt, kind="ExternalInput")
    d_ws = nc.dram_tensor("ws", [NBLK, D2, HID], dt, kind="ExternalInput")
    d_wc = nc.dram_tensor("wc", [HID, HID], dt, kind="ExternalInput")
    d_bias = nc.dram_tensor("bias", [128, NBIAS], dt, kind="ExternalInput")
    d_out = nc.dram_tensor("out_pts", [128, NPTS], dt, kind="ExternalOutput")
    d_scr = nc.dram_tensor("pt_scratch", [4, NPTS], dt)   # internal scratch

    with tile.TileContext(nc) as tc, ExitStack() as ctx:
        cpool = ctx.enter_context(tc.tile_pool(name="const", bufs=1))
        spool = ctx.enter_context(tc.tile_pool(name="stage", bufs=2))
        psumM = ctx.enter_context(tc.tile_pool(name="psumM", bufs=4, space="PSUM"))
        psumN = ctx.enter_context(tc.tile_pool(name="psumN", bufs=4, space="PSUM"))

        # ---------------- persistent sbuf
        net = cpool.tile([128, NPTS], dt, tag="net")
        mall = cpool.tile([128, NPTS + 16], dh, tag="mall")
        bias = cpool.tile([128, NBIAS], dt, tag="bias")
        wpos = cpool.tile([4, D2], dt, tag="wpos")
        w0a = [cpool.tile([128, HID], dt, tag=f"w0a{i}", name=f"w0a{i}") for i in range(NBLK)]
        w0b = [cpool.tile([128, HID], dt, tag=f"w0b{i}", name=f"w0b{i}") for i in range(NBLK)]
        w1 = [cpool.tile([128, HID], dt, tag=f"w1{i}", name=f"w1{i}") for i in range(NBLK)]
        wsa = [cpool.tile([128, HID], dt, tag=f"wsa{i}", name=f"wsa{i}") for i in range(NBLK)]
        wsb = [cpool.tile([128, HID], dt, tag=f"wsb{i}", name=f"wsb{i}") for i in range(NBLK)]
        wc = cpool.tile([128, HID], dt, tag="wc")

        nc.sync.dma_start(mall[:], d_m[0:1, :].to_broadcast((128, NPTS + 16)))
        nc.sync.dma_start(bias[:], d_bias[:])
        nc.sync.dma_start(wpos[:], d_wpos4[:])
        for i in range(NBLK):
            nc.sync.dma_start(w0a[i][:], d_w0[i, 0:128, :])
            nc.sync.dma_start(w0b[i][:], d_w0[i, 128:256, :])
            nc.sync.dma_start(w1[i][:], d_w1[i, :, :])
            nc.sync.dma_start(wsa[i][:], d_ws[i, 0:128, :])
            nc.sync.dma_start(wsb[i][:], d_ws[i, 128:256, :])
        nc.sync.dma_start(wc[:], d_wc[:])

        # ---------------- pt = 2*frac(clip(p+.5)*res) - 1, flat layout
        pflat = spool.tile([128, FW], dt, tag="pre", bufs=1, name="pflat")
        nc.sync.dma_start(pflat[:], d_pts[:])
        nc.vector.tensor_scalar(pflat[:], pflat[:], 0.5, 1.0 - 1e-6, OP.add, OP.min)
        nc.vector.tensor_scalar(pflat[:], pflat[:], 1e-6, float(RES), OP.max, OP.mult)
        ci = spool.tile([128, FW], mybir.dt.int16, tag="pre2", bufs=1, name="ci")
        nc.vector.tensor_copy(ci[:], pflat[:])
        nc.vector.tensor_tensor(pflat[:], pflat[:], ci[:], OP.subtract)
        nc.vector.scalar_tensor_tensor(pflat[:], pflat[:], 0.0, pflat[:],
                                       OP.is_lt, OP.add)
        nc.vector.tensor_scalar(pflat[:], pflat[:], 2.0, -1.0, OP.mult, OP.add)
        scr_flat = d_scr[:].rearrange("a (b f) -> (a b) f", f=FW)
        nc.sync.dma_start(scr_flat, pflat[:])

        def evac(dst, src, bias_col=None, gelu=False, eng="act"):
            if eng == "act":
                f = GELU if gelu else (
                    AF.Identity if bias_col is not None else AF.Copy)
                nc.scalar.activation(
                    dst, src, f,
                    bias=bias_col if bias_col is not None else 0.0)
            else:
                assert not gelu
                if bias_col is not None:
                    nc.vector.tensor_scalar(dst, src, bias_col, None, OP.add)
                else:
                    nc.vector.tensor_copy(dst, src)

        # ---------------- setup: pos-mlp + resblock 0, per 512-chunk
        for c in range(NCHUNK):
            ptc = spool.tile([4, 512], dt, tag="ptc", bufs=1, name="ptc")
            nc.sync.dma_start(ptc[:], d_scr[:, c * 512:(c + 1) * 512])
            x0a = psumM.tile([128, 512], dt, tag="mm")
            x0b = psumN.tile([128, 512], dt, tag="nn", name="x0b")
            nc.tensor.matmul(x0a[:], wpos[:, 0:128], ptc[:], start=True, stop=True)
            nc.tensor.matmul(x0b[:], wpos[:, 128:256], ptc[:], start=True, stop=True)
            gxa = spool.tile([128, 512], dt, tag="gpool", bufs=1, name="gxa")
            gxb = spool.tile([128, 512], dt, tag="gnet", bufs=1, name="gxb")
            rxa = spool.tile([128, 512], dt, tag="fs", bufs=7, name="rxa")
            rxb = spool.tile([128, 512], dt, tag="tot", bufs=3, name="rxb")
            evac(gxa[:], x0a[:], bias[:, 0:1], gelu=True)
            evac(gxb[:], x0b[:], bias[:, 1:2], gelu=True)
            evac(rxa[:], x0a[:], bias[:, 0:1], eng="dve")
            evac(rxb[:], x0b[:], bias[:, 1:2], eng="dve")
            hp = psumM.tile([128, 512], dt, tag="mm", name="hp0")
            npp = psumN.tile([128, 512], dt, tag="nn", name="npp0")
            nc.tensor.matmul(hp[:], w0a[0][:], gxa[:], start=True, stop=False)
            nc.tensor.matmul(npp[:], wsa[0][:], rxa[:], start=True, stop=False)
            nc.tensor.matmul(hp[:], w0b[0][:], gxb[:], start=False, stop=True)
            nc.tensor.matmul(npp[:], wsb[0][:], rxb[:], start=False, stop=False)
            gh = spool.tile([128, 512], dt, tag="ghs", bufs=1, name="gh0")
            evac(gh[:], hp[:], bias[:, 2:3], gelu=True)
            nc.tensor.matmul(npp[:], w1[0][:], gh[:], start=False, stop=True)
            evac(net[:, c * 512:(c + 1) * 512], npp[:], bias[:, 7:8], eng="dve")

        # ---------------- segmented mean per region: fs, *=emrp, reverse hold
        def pooled_region(r0, r1, src_of_chunk, want_tot=True):
            """Returns list of (c, tot_tile) for chunks [r0, r1)."""
            fss = {}
            for c in range(r0, r1):
                src = src_of_chunk(c)
                fs = spool.tile([128, 512], dt, tag="fs", bufs=7, name="fs")
                init = 0.0 if c == r0 else fss[c - 1][:, 511:512]
                nc.vector.tensor_tensor_scan(
                    fs[:], mall[:, c * 512:(c + 1) * 512], src, init,
                    OP.mult, OP.add)
                fss[c] = fs
            for c in range(r0, r1):
                emr = spool.tile([128, 512], dt, tag="emr", bufs=2, name="emr")
                nc.sync.dma_start(emr[:], d_emr[c:c + 1, :].to_broadcast((128, 512)))
                nc.vector.tensor_tensor(fss[c][:], fss[c][:], emr[:], OP.mult)
            if not want_tot:
                return [(c, fss[c]) for c in range(r0, r1)]
            tots = {}
            for c in range(r1 - 1, r0 - 1, -1):
                tot = spool.tile([128, 512], dt, tag="tot", bufs=3, name="tot")
                init = 0.0 if c == r1 - 1 else tots[c + 1][:, 0:1]
                # h = m shifted left by one; reversed APs give a backward scan
                h_rev = mall[:, c * 512 + 512:c * 512:-1]
                nc.vector.tensor_tensor_scan(
                    tot[:, ::-1], h_rev, fss[c][:, ::-1], init,
                    OP.mult, OP.add)
                tots[c] = tot
            return [(c, tots[c]) for c in range(r0, r1)]

        # ---------------- pooling iterations
        for i in range(1, NBLK):
            for (r0, r1) in rbounds:
                pairs = pooled_region(r0, r1, lambda c: net[:, c * 512:(c + 1) * 512])
                for c, tot in reversed(pairs):
                    ns = slice(c * 512, (c + 1) * 512)
                    gpool = spool.tile([128, 512], dt, tag="gpool", bufs=1)
                    gnet = spool.tile([128, 512], dt, tag="gnet", bufs=1)
                    evac(gpool[:], tot[:], gelu=True)
                    evac(gnet[:], net[:, ns], gelu=True)
                    hp = psumM.tile([128, 512], dt, tag="mm", name="hpi")
                    npp = psumN.tile([128, 512], dt, tag="nn", name="nppi")
                    nc.tensor.matmul(hp[:], w0a[i][:], gnet[:], start=True, stop=False)
                    nc.tensor.matmul(npp[:], wsa[i][:], net[:, ns], start=True, stop=False)
                    nc.tensor.matmul(hp[:], w0b[i][:], gpool[:], start=False, stop=True)
                    nc.tensor.matmul(npp[:], wsb[i][:], tot[:], start=False, stop=False)
                    gh = spool.tile([128, 512], dt, tag="ghs", bufs=1, name="ghi")
                    evac(gh[:], hp[:], bias[:, 2 + i:3 + i], gelu=True)
                    nc.tensor.matmul(npp[:], w1[i][:], gh[:], start=False, stop=True)
                    evac(net[:, ns], npp[:], bias[:, 7 + i:8 + i], eng="act")

        # ---------------- head: c = net @ Wc + b_c, masked segment means out
        def head_src(c):
            cp = psumM.tile([128, 512], dt, tag="mm", name="cp")
            nc.tensor.matmul(cp[:], wc[:], net[:, c * 512:(c + 1) * 512],
                             start=True, stop=True)
            cv = spool.tile([128, 512], dt, tag="gpool", bufs=1, name="cv")
            evac(cv[:], cp[:], bias[:, 12:13], eng="act")
            return cv[:]

        for (r0, r1) in rbounds:
            outs = pooled_region(r0, r1, head_src, want_tot=False)
            for c, g in outs:
                nc.sync.dma_start(d_out[:, c * 512:(c + 1) * 512], g[:])

    return nc


# ================================================================ run + glue
_BUILT = {}


def get_nc():
    if "nc" not in _BUILT:
        nc = build_bass()
        nc.compile()
        _BUILT["nc"] = nc
    return _BUILT["nc"]


def make_in_maps(p, sparse_coords, W_pos, b_pos, W0, b0, W1, b1, Ws, Wc, b_c, res):
    index, counts = point_meta(p, sparse_coords, int(res))
    shards = shard(np.asarray(p, F32), index)
    wdict = weight_inputs(W_pos, b_pos, W0, b0, W1, b1, Ws, Wc, b_c)
    in_maps = []
    for sh in shards:
        ci = core_inputs(sh)
        sh["end_pos"] = ci["end_pos"]
        m = dict(pts_flat=ci["pts_flat"], m_row=ci["m_row"], emrp=ci["emrp"],
                 wpos4=wdict["wpos4"], w0=wdict["w0"], w1=wdict["w1"],
                 ws=wdict["ws"], wc=wdict["wc"], bias=wdict["bias"])
        in_maps.append(m)
    return in_maps, shards, counts


def assemble(results, shards, counts, sparse_coords):
    sc = np.asarray(sparse_coords)
    starts = np.concatenate([[0], np.cumsum(counts)[:-1]])
    out = np.zeros((sc.shape[0], HID), F32)
    for sh, r_ in zip(shards, results):
        g = np.asarray(r_["out_pts"])                 # [128, NPTS] masked means
        lo, b = sh["lo"], sh["batch"]
        row0 = starts[b] + lo
        for rb, pos in sh["end_pos"].items():
            out[row0 + rb] = g[:, pos]
    return out


def kernel(p, sparse_coords, W_pos, b_pos, W0, b0, W1, b1, Ws, Wc, b_c, res):
    if "/opt/trn_rl_repo" not in sys.path:
        sys.path.insert(0, "/opt/trn_rl_repo")
    from concourse.bass_utils import run_bass_kernel_spmd

    in_maps, shards, counts = make_in_maps(
        p, sparse_coords, W_pos, b_pos, W0, b0, W1, b1, Ws, Wc, b_c, res)
    nc = get_nc()
    results = run_bass_kernel_spmd(nc, in_maps, list(range(NCORES))).results
    return assemble(results, shards, counts, sparse_coords)


# revision 19
# speedup vs baseline: 1.0611x; 1.0611x over previous
"""LocalPoolPointnet on 8 Trainium2 cores (v5: segmented scans, no one-hot).

The data has ~124 occupied bins per core with 30-800 points each (points
sorted by bin).  scatter_mean + gather become three DVE passes per tile:

  fs  = tensor_tensor_scan(m, net)        # segmented running sum (m: 0 at
                                          # segment starts, else 1)
  fs *= em_rp                             # keep mean (= sum * 1/cnt) at each
                                          # segment END, zero elsewhere
  tot = reverse-scan(m>>1, fs)            # broadcast each segment's mean
                                          # back over the segment (hold scan)

All matmuls are plain fp32 resblock GEMMs; there is no bin table, no one-hot,
no transposes, no dynamic addressing on the device.  Points are packed into
static REGIONS (whole bins per region, dummy-padded); scans chain inside a
region via static initial wiring and the masks kill any cross-region carry.
The head emits masked means in point space; the host gathers segment ends.
"""

import sys
import numpy as np

# ---------------------------------------------------------------- constants
B = 2
NP_ = 100_000
HID = 128
D2 = 256
NBLK = 5
RES = 64
R = 20_005          # max_coord_num in the reference
BIG = RES ** 3 + 1
NCORES = 8
CORES_PER_BATCH = NCORES // B

NCHUNK = 52                     # 512-pt chunks per core
NPTS = NCHUNK * 512             # 26624 padded points per core
REGIONS = [6, 6, 6, 6, 6, 6, 6, 6, 4]   # chunks per region (sum = 52)
assert sum(REGIONS) == NCHUNK
NBIAS = 13                      # b_pos(2) b0(5) b1(5) b_c(1)
FW = NPTS * 4 // 128            # 832

F32 = np.float32
F16 = np.float16


# ================================================================ host prep
def point_meta(p, sparse_coords, res):
    """Integer routing metadata, bit-identical to the reference's indexing."""
    p = np.asarray(p, F32)
    sc = np.asarray(sparse_coords)
    coord = np.clip(p + F32(0.5), F32(1e-6), F32(1.0 - 1e-6)) * F32(res)
    cl = coord.astype(np.int32)
    lin = (cl[..., 0] * res + cl[..., 1]) * res + cl[..., 2]      # [B, NP]

    slin = (sc[:, 1] * res + sc[:, 2]) * res + sc[:, 3]
    index = np.empty((B, NP_), np.int64)
    for b in range(B):
        coords_b = np.sort(np.where(sc[:, 0] == b, slin, BIG))
        index[b] = np.searchsorted(coords_b, lin[b], side="left")
    counts = np.bincount(sc[:, 0], minlength=B)
    return index, counts


def shard(p, index):
    """Split each batch's points into CORES_PER_BATCH contiguous-bin shards."""
    shards = []
    for b in range(B):
        idx = index[b]
        order = np.argsort(idx, kind="stable")
        sidx = idx[order]
        binc = np.bincount(idx, minlength=R)
        csum = np.cumsum(binc)
        prev_hi = 0
        for c in range(CORES_PER_BATCH):
            if c < CORES_PER_BATCH - 1:
                target = (c + 1) * NP_ // CORES_PER_BATCH
                hi = int(np.searchsorted(csum, target))
                if hi > 0 and target - csum[hi - 1] < csum[hi] - target:
                    hi -= 1
                hi += 1          # shard owns bins [lo, hi)
            else:
                hi = R
            lo = prev_hi
            prev_hi = hi
            sel = slice(int(np.searchsorted(sidx, lo)), int(np.searchsorted(sidx, hi)))
            pts = p[b][order[sel]]                     # [n, 3] sorted by bin
            rel = (sidx[sel] - lo).astype(np.int64)    # sorted rel bins
            shards.append(dict(batch=b, lo=lo, hi=hi, pts=pts, rel=rel,
                               nb=hi - lo))
    return shards


def core_inputs(sh):
    """Whole-bin region packing + scan masks for one core."""
    n = sh["pts"].shape[0]
    rel = sh["rel"]

    pts = np.full((NPTS, 3), 0.25, F32)
    m = np.ones(NPTS + 16, F32)          # scan carry mask (0 = segment start)
    emrp = np.zeros(NPTS, F32)           # 1/cnt at segment ends, else 0
    end_pos = {}                         # rel bin -> padded end position

    # bin run boundaries
    starts = np.flatnonzero(np.r_[True, rel[1:] != rel[:-1]])
    ends = np.r_[starts[1:], n]
    nbin = len(starts)

    region_caps = [r * 512 for r in REGIONS]
    # best-fit-decreasing bin packing into regions (bin order is free)
    order = sorted(range(nbin), key=lambda k: -(ends[k] - starts[k]))
    left = list(region_caps)
    fill = [0] * len(region_caps)
    rbase = np.cumsum([0] + region_caps[:-1])
    assign = {}
    for k in order:
        ln = int(ends[k] - starts[k])
        cands = [r for r in range(len(left)) if left[r] >= ln]
        assert cands, f"bin of {ln} pts does not fit any region"
        r = min(cands, key=lambda r_: left[r_])
        assign[k] = r
        left[r] -= ln
    for k in range(nbin):
        r = assign[k]
        s, e = starts[k], ends[k]
        ln = e - s
        pos = int(rbase[r]) + fill[r]
        pts[pos:pos + ln] = sh["pts"][s:e]
        m[pos] = 0.0
        emrp[pos + ln - 1] = F32(1.0) / F32(ln)
        end_pos[int(rel[s])] = pos + ln - 1
        fill[r] += ln
    m[0] = 0.0
    # region starts always begin a new segment (kills static scan carry)
    off = 0
    for cap in region_caps:
        m[off] = 0.0
        off += cap

    cnt = np.bincount(rel, minlength=max(sh["nb"], 1)).astype(F32)

    pts4 = np.zeros((4, NPTS), F32)
    pts4[:3] = pts.T
    pts_flat = np.ascontiguousarray(pts4).reshape(128, FW)
    return dict(pts_flat=pts_flat, m_row=m[None, :].astype(F16),
                emrp=emrp.reshape(NCHUNK, 512), cnt=cnt, end_pos=end_pos)


def weight_inputs(W_pos, b_pos, W0, b0, W1, b1, Ws, Wc, b_c):
    W_pos, W0, W1, Ws, Wc = [np.ascontiguousarray(x, F32)
                             for x in (W_pos, W0, W1, Ws, Wc)]
    wpos4 = np.zeros((4, D2), F32)
    wpos4[:3] = W_pos
    bias = np.zeros((128, NBIAS), F32)
    bias[:, 0] = np.asarray(b_pos, F32)[:128]
    bias[:, 1] = np.asarray(b_pos, F32)[128:]
    bias[:, 2:7] = np.asarray(b0, F32).T
    bias[:, 7:12] = np.asarray(b1, F32).T
    bias[:, 12] = np.asarray(b_c, F32)
    return dict(wpos4=wpos4, w0=W0, w1=W1, ws=Ws, wc=Wc, bias=bias)


# ================================================================ bass build
def build_bass():
    if "/opt/trn_rl_repo" not in sys.path:
        sys.path.insert(0, "/opt/trn_rl_repo")
    import concourse.mybir as mybir
    from concourse import bacc, tile
    from contextlib import ExitStack

    dt = mybir.dt.float32
    dh = mybir.dt.float16
    AF = mybir.ActivationFunctionType
    OP = mybir.AluOpType
    GELU = AF.Gelu_apprx_tanh

    # region layout in chunks
    rbounds = []
    off = 0
    for r in REGIONS:
        rbounds.append((off, off + r))
        off += r

    nc = bacc.Bacc("TRN2")
    # -------- dram io
    d_pts = nc.dram_tensor("pts_flat", [128, FW], dt, kind="ExternalInput")
    d_m = nc.dram_tensor("m_row", [1, NPTS + 16], dh, kind="ExternalInput")
    d_emr = nc.dram_tensor("emrp", [NCHUNK, 512], dt, kind="ExternalInput")
    d_wpos4 = nc.dram_tensor("wpos4", [4, D2], dt, kind="ExternalInput")
    d_w0 = nc.dram_tensor("w0", [NBLK, D2, HID], dt, kind="ExternalInput")
    d_w1 = nc.dram_tensor("w1", [NBLK, HID, HID], dt, kind="ExternalInput")
    d_ws = nc.dram_tensor("ws", [NBLK, D2, HID], dt, kind="ExternalInput")
    d_wc = nc.dram_tensor("wc", [HID, HID], dt, kind="ExternalInput")
    d_bias = nc.dram_tensor("bias", [128, NBIAS], dt, kind="ExternalInput")
    d_out = nc.dram_tensor("out_pts", [128, NPTS], dt, kind="ExternalOutput")
    d_scr = nc.dram_tensor("pt_scratch", [4, NPTS], dt)   # internal scratch

    with tile.TileContext(nc) as tc, ExitStack() as ctx:
        cpool = ctx.enter_context(tc.tile_pool(name="const", bufs=1))
        spool = ctx.enter_context(tc.tile_pool(name="stage", bufs=2))
        psumM = ctx.enter_context(tc.tile_pool(name="psumM", bufs=4, space="PSUM"))
        psumN = ctx.enter_context(tc.tile_pool(name="psumN", bufs=4, space="PSUM"))

        # ---------------- persistent sbuf
        net = cpool.tile([128, NPTS], dt, tag="net")
        mall = cpool.tile([128, NPTS + 16], dh, tag="mall")
        bias = cpool.tile([128, NBIAS], dt, tag="bias")
        wpos = cpool.tile([4, D2], dt, tag="wpos")
        w0a = [cpool.tile([128, HID], dt, tag=f"w0a{i}", name=f"w0a{i}") for i in range(NBLK)]
        w0b = [cpool.tile([128, HID], dt, tag=f"w0b{i}", name=f"w0b{i}") for i in range(NBLK)]
        w1 = [cpool.tile([128, HID], dt, tag=f"w1{i}", name=f"w1{i}") for i in range(NBLK)]
        wsa = [cpool.tile([128, HID], dt, tag=f"wsa{i}", name=f"wsa{i}") for i in range(NBLK)]
        wsb = [cpool.tile([128, HID], dt, tag=f"wsb{i}", name=f"wsb{i}") for i in range(NBLK)]
        wc = cpool.tile([128, HID], dt, tag="wc")

        nc.sync.dma_start(mall[:], d_m[0:1, :].to_broadcast((128, NPTS + 16)))
        nc.sync.dma_start(bias[:], d_bias[:])
        nc.sync.dma_start(wpos[:], d_wpos4[:])
        for i in range(NBLK):
            nc.sync.dma_start(w0a[i][:], d_w0[i, 0:128, :])
            nc.sync.dma_start(w0b[i][:], d_w0[i, 128:256, :])
            nc.sync.dma_start(w1[i][:], d_w1[i, :, :])
            nc.sync.dma_start(wsa[i][:], d_ws[i, 0:128, :])
            nc.sync.dma_start(wsb[i][:], d_ws[i, 128:256, :])
        nc.sync.dma_start(wc[:], d_wc[:])

        # ---------------- pt = 2*frac(clip(p+.5)*res) - 1, flat layout
        pflat = spool.tile([128, FW], dt, tag="pre", bufs=1, name="pflat")
        nc.sync.dma_start(pflat[:], d_pts[:])
        nc.vector.tensor_scalar(pflat[:], pflat[:], 0.5, 1.0 - 1e-6, OP.add, OP.min)
        nc.vector.tensor_scalar(pflat[:], pflat[:], 1e-6, float(RES), OP.max, OP.mult)
        ci = spool.tile([128, FW], mybir.dt.int16, tag="pre2", bufs=1, name="ci")
        nc.vector.tensor_copy(ci[:], pflat[:])
        nc.vector.tensor_tensor(pflat[:], pflat[:], ci[:], OP.subtract)
        nc.vector.scalar_tensor_tensor(pflat[:], pflat[:], 0.0, pflat[:],
                                       OP.is_lt, OP.add)
        nc.vector.tensor_scalar(pflat[:], pflat[:], 2.0, -1.0, OP.mult, OP.add)
        scr_flat = d_scr[:].rearrange("a (b f) -> (a b) f", f=FW)
        nc.sync.dma_start(scr_flat, pflat[:])

        def evac(dst, src, bias_col=None, gelu=False, eng="act"):
            if eng == "act":
                f = GELU if gelu else (
                    AF.Identity if bias_col is not None else AF.Copy)
                nc.scalar.activation(
                    dst, src, f,
                    bias=bias_col if bias_col is not None else 0.0)
            else:
                assert not gelu
                if bias_col is not None:
                    nc.vector.tensor_scalar(dst, src, bias_col, None, OP.add)
                else:
                    nc.vector.tensor_copy(dst, src)

        # ---------------- setup: pos-mlp + resblock 0, per 512-chunk
        for c in range(NCHUNK):
            ptc = spool.tile([4, 512], dt, tag="ptc", bufs=1, name="ptc")
            nc.sync.dma_start(ptc[:], d_scr[:, c * 512:(c + 1) * 512])
            x0a = psumM.tile([128, 512], dt, tag="mm")
            x0b = psumN.tile([128, 512], dt, tag="nn", name="x0b")
            nc.tensor.matmul(x0a[:], wpos[:, 0:128], ptc[:], start=True, stop=True)
            nc.tensor.matmul(x0b[:], wpos[:, 128:256], ptc[:], start=True, stop=True)
            gxa = spool.tile([128, 512], dt, tag="gpool", bufs=1, name="gxa")
            gxb = spool.tile([128, 512], dt, tag="gnet", bufs=1, name="gxb")
            rxa = spool.tile([128, 512], dt, tag="fs", bufs=7, name="rxa")
            rxb = spool.tile([128, 512], dt, tag="tot", bufs=3, name="rxb")
            evac(gxa[:], x0a[:], bias[:, 0:1], gelu=True)
            evac(gxb[:], x0b[:], bias[:, 1:2], gelu=True)
            evac(rxa[:], x0a[:], bias[:, 0:1], eng="dve")
            evac(rxb[:], x0b[:], bias[:, 1:2], eng="dve")
            hp = psumM.tile([128, 512], dt, tag="mm", name="hp0")
            nc.tensor.matmul(hp[:], w0a[0][:], gxa[:], start=True, stop=False)
            nc.tensor.matmul(hp[:], w0b[0][:], gxb[:], start=False, stop=True)
            npp = psumN.tile([128, 512], dt, tag="nn", name="npp0")
            nc.tensor.matmul(npp[:], wsa[0][:], rxa[:], start=True, stop=False)
            nc.tensor.matmul(npp[:], wsb[0][:], rxb[:], start=False, stop=False)
            gh = spool.tile([128, 512], dt, tag="ghs", bufs=1, name="gh0")
            evac(gh[:], hp[:], bias[:, 2:3], gelu=True)
            nc.tensor.matmul(npp[:], w1[0][:], gh[:], start=False, stop=True)
            evac(net[:, c * 512:(c + 1) * 512], npp[:], bias[:, 7:8], eng="dve")

        # ---------------- segmented mean per region: fs, *=emrp, reverse hold
        def pooled_region(r0, r1, src_of_chunk, want_tot=True):
            """Returns list of (c, tot_tile) for chunks [r0, r1)."""
            fss = {}
            for c in range(r0, r1):
                src = src_of_chunk(c)
                fs = spool.tile([128, 512], dt, tag="fs", bufs=7, name="fs")
                init = 0.0 if c == r0 else fss[c - 1][:, 511:512]
                nc.vector.tensor_tensor_scan(
                    fs[:], mall[:, c * 512:(c + 1) * 512], src, init,
                    OP.mult, OP.add)
                fss[c] = fs
            for c in range(r0, r1):
                emr = spool.tile([128, 512], dt, tag="emr", bufs=2, name="emr")
                nc.sync.dma_start(emr[:], d_emr[c:c + 1, :].to_broadcast((128, 512)))
                nc.vector.tensor_tensor(fss[c][:], fss[c][:], emr[:], OP.mult)
            if not want_tot:
                return [(c, fss[c]) for c in range(r0, r1)]
            tots = {}
            for c in range(r1 - 1, r0 - 1, -1):
                tot = spool.tile([128, 512], dt, tag="tot", bufs=3, name="tot")
                init = 0.0 if c == r1 - 1 else tots[c + 1][:, 0:1]
                # h = m shifted left by one; reversed APs give a backward scan
                h_rev = mall[:, c * 512 + 512:c * 512:-1]
                nc.vector.tensor_tensor_scan(
                    tot[:, ::-1], h_rev, fss[c][:, ::-1], init,
                    OP.mult, OP.add)
                tots[c] = tot
            return [(c, tots[c]) for c in range(r0, r1)]

        # ---------------- pooling iterations
        for i in range(1, NBLK):
            for (r0, r1) in rbounds:
                pairs = pooled_region(r0, r1, lambda c: net[:, c * 512:(c + 1) * 512])
                for c, tot in reversed(pairs):
                    ns = slice(c * 512, (c + 1) * 512)
                    gpool = spool.tile([128, 512], dt, tag="gpool", bufs=1)
                    gnet = spool.tile([128, 512], dt, tag="gnet", bufs=1)
                    evac(gpool[:], tot[:], gelu=True)
                    evac(gnet[:], net[:, ns], gelu=True)
                    hp = psumM.tile([128, 512], dt, tag="mm", name="hpi")
                    nc.tensor.matmul(hp[:], w0a[i][:], gnet[:], start=True, stop=False)
                    nc.tensor.matmul(hp[:], w0b[i][:], gpool[:], start=False, stop=True)
                    npp = psumN.tile([128, 512], dt, tag="nn", name="nppi")
                    nc.tensor.matmul(npp[:], wsa[i][:], net[:, ns], start=True, stop=False)
                    nc.tensor.matmul(npp[:], wsb[i][:], tot[:], start=False, stop=False)
                    gh = spool.tile([128, 512], dt, tag="ghs", bufs=1, name="ghi")
                    evac(gh[:], hp[:], bias[:, 2 + i:3 + i], gelu=True)
                    nc.tensor.matmul(npp[:], w1[i][:], gh[:], start=False, stop=True)
                    evac(net[:, ns], npp[:], bias[:, 7 + i:8 + i], eng="act")

        # ---------------- head: c = net @ Wc + b_c, masked segment means out
        def head_src(c):
            cp = psumM.tile([128, 512], dt, tag="mm", name="cp")
            nc.tensor.matmul(cp[:], wc[:], net[:, c * 512:(c + 1) * 512],
                             start=True, stop=True)
            cv = spool.tile([128, 512], dt, tag="gpool", bufs=1, name="cv")
            evac(cv[:], cp[:], bias[:, 12:13], eng="act")
            return cv[:]

        for (r0, r1) in rbounds:
            outs = pooled_region(r0, r1, head_src, want_tot=False)
            for c, g in outs:
                nc.sync.dma_start(d_out[:, c * 512:(c + 1) * 512], g[:])

    return nc


# ================================================================ run + glue
_BUILT = {}


def get_nc():
    if "nc" not in _BUILT:
        nc = build_bass()
        nc.compile()
        _BUILT["nc"] = nc
    return _BUILT["nc"]


def make_in_maps(p, sparse_coords, W_pos, b_pos, W0, b0, W1, b1, Ws, Wc, b_c, res):
    index, counts = point_meta(p, sparse_coords, int(res))
    shards = shard(np.asarray(p, F32), index)
    wdict = weight_inputs(W_pos, b_pos, W0, b0, W1, b1, Ws, Wc, b_c)
    in_maps = []
    for sh in shards:
        ci = core_inputs(sh)
        sh["end_pos"] = ci["end_pos"]
        m = dict(pts_flat=ci["pts_flat"], m_row=ci["m_row"], emrp=ci["emrp"],
                 wpos4=wdict["wpos4"], w0=wdict["w0"], w1=wdict["w1"],
                 ws=wdict["ws"], wc=wdict["wc"], bias=wdict["bias"])
        in_maps.append(m)
    return in_maps, shards, counts


def assemble(results, shards, counts, sparse_coords):
    sc = np.asarray(sparse_coords)
    starts = np.concatenate([[0], np.cumsum(counts)[:-1]])
    out = np.zeros((sc.shape[0], HID), F32)
    for sh, r_ in zip(shards, results):
        g = np.asarray(r_["out_pts"])                 # [128, NPTS] masked means
        lo, b = sh["lo"], sh["batch"]
        row0 = starts[b] + lo
        for rb, pos in sh["end_pos"].items():
            out[row0 + rb] = g[:, pos]
    return out


def kernel(p, sparse_coords, W_pos, b_pos, W0, b0, W1, b1, Ws, Wc, b_c, res):
    if "/opt/trn_rl_repo" not in sys.path:
        sys.path.insert(0, "/opt/trn_rl_repo")
    from concourse.bass_utils import run_bass_kernel_spmd

    in_maps, shards, counts = make_in_maps(
        p, sparse_coords, W_pos, b_pos, W0, b0, W1, b1, Ws, Wc, b_c, res)
    nc = get_nc()
    results = run_bass_kernel_spmd(nc, in_maps, list(range(NCORES))).results
    return assemble(results, shards, counts, sparse_coords)
